# revision 44
# baseline (speedup 1.0000x reference)
"""Distributed Trainium2 kernel for nn_Attention (B=2,S=4096,D=2048,H=16).

Tensor-parallel over heads across 8 NeuronCores; core c owns heads 2c,2c+1.

Host prep (free): x -> xT [D, B*S] fp16; per-core wq/wk/wv column slices
pre-transposed, with rotary pair de-interleave folded into the wq/wk row
permutation; rotary cos/sin combined with the RMS-norm weights into 4
coefficient planes; wo pre-transposed.

Per core:
  1. QKV over 512-wide s-chunks (x DMA split across both HWDGE queues,
     triple-buffered, so the PE never waits and HAM stays warm). RMS-norm
     partition-reduction via a ones matmul into a [128,512] broadcast
     layout; 1/sqrt(var) computed as exp(-0.5*ln(var)) on ScalarE (keeps
     the slow DVE reciprocal off the dependency chain); rotary on VectorE;
     v PE-transposed to natural [s, hd+1] layout with an appended ones
     column (copies on DVE). Epilogues software-pipelined one matmul group
     behind.
  2. Attention per (b, head, 512-wide q block): scoresT = kT.T @ qT with
     N=512 matmuls (LDWEIGHTS fully hidden), exp on ScalarE on [128,2,512]
     score pairs straight out of PSUM (bf16 out; scores bounded so no
     max-subtraction), PV accumulates probsT.T @ [v|1] into a [128,4,256]
     PSUM tile (two 129-wide accumulators per bank) giving attention output
     and softmax row sums in one pass. PV emitted 2 exp-pairs behind QK.
  3. Output ownership is stride-4 interleaved: core j owns q-tiles
     {4k + j%4} of batch j//4. Each of 4 passes computes q-blocks 2t,2t+1
     for every (b,h), AllToAll's one 256-col chunk per dest, and the output
     projection for the previous pass runs behind it with wo streamed from
     DRAM in 2MB pieces (no bulk 8MB stall).
Host reassembles the interleaved row blocks.
"""
import sys

sys.path.insert(0, "/opt/trn_rl_repo")

import numpy as np
import ml_dtypes

import concourse.bass as bass
import concourse.bacc as bacc
import concourse.mybir as mybir
import concourse.tile as tile
from concourse import masks
from concourse.bass_utils import run_bass_kernel_spmd

DT16 = mybir.dt.float16
BF16 = mybir.dt.bfloat16
F32 = mybir.dt.float32

B, S, D, H = 2, 4096, 2048, 16
HD = 128                  # head dim
NCORES = 8
HPC = H // NCORES         # heads per core = 2
BS = B * S                # 8192
KC = D // 128             # 16 contraction chunks
SCH = 512                 # s-chunk for QKV phase
NSCH = BS // SCH          # 16
SLICE = BS // NCORES      # 1024 output rows per core
NT = 4                    # attention/a2a passes
EPS = 1e-5
ISQ = 1.0 / np.sqrt(HD)

_CACHE = {}


def _build():
    nc = bacc.Bacc("TRN2", target_bir_lowering=False, debug=False,
                   num_devices=NCORES)

    xt = nc.dram_tensor("xt", [D, BS], DT16, kind="ExternalInput")
    wqt = nc.dram_tensor("wqt", [D, HPC * HD], DT16, kind="ExternalInput")
    wkt = nc.dram_tensor("wkt", [D, HPC * HD], DT16, kind="ExternalInput")
    wvt = nc.dram_tensor("wvt", [D, HPC * HD], DT16, kind="ExternalInput")
    wot = nc.dram_tensor("wot", [D, D], DT16, kind="ExternalInput")
    # plane 0 rows = [A(64); B(64)], plane 1 rows = [C(64); D(64)] so every
    # rotary multiply pairs SBUF operands with equal base partition.
    rq = nc.dram_tensor("rq", [2, 128, BS], DT16, kind="ExternalInput")
    rk = nc.dram_tensor("rk", [2, 128, BS], DT16, kind="ExternalInput")
    out_ext = nc.dram_tensor("out", [SLICE, D], F32, kind="ExternalOutput")

    with tile.TileContext(nc) as tc:
        with tc.tile_pool(name="persist", bufs=1) as pp, \
             tc.tile_pool(name="dramp", bufs=1, space="DRAM") as dramp:
            ident = pp.tile([128, 128], DT16)
            masks.make_identity(nc, ident[:])
            ones_sq = pp.tile([128, 128], DT16)
            nc.gpsimd.memset(ones_sq[:], 1.0)
            eps_t = pp.tile([128, 1], F32)
            nc.gpsimd.memset(eps_t[:], EPS)

            # per-head tensors living through phases 1-2
            qkvp = tc.alloc_tile_pool(name="qkvp", bufs=1)
            q_sb = [qkvp.tile([128, BS], DT16, name=f"q{h}")
                    for h in range(HPC)]
            k_sb = [qkvp.tile([128, BS], DT16, name=f"k{h}")
                    for h in range(HPC)]
            # v in natural layout per 128-row s-tile, ones column at 128
            v_sb = [qkvp.tile([128, BS // 128, HD + 1], BF16, name=f"v{h}")
                    for h in range(HPC)]
            for h in range(HPC):
                nc.gpsimd.memset(v_sb[h][:, :, HD:HD + 1], 1.0)

            # ---------------- Phase 1: QKV + RMS + rotary ----------------
            with tc.tile_pool(name="p1", bufs=1) as p1, \
                 tc.tile_pool(name="p1ps", bufs=1,
                              space=bass.MemorySpace.PSUM) as p1ps:
                wq_s = p1.tile([128, KC, HPC * HD], DT16)
                wk_s = p1.tile([128, KC, HPC * HD], DT16)
                wv_s = p1.tile([128, KC, HPC * HD], DT16)

                def load_w(wdst, wsrc, eng):
                    wr = wsrc.ap().rearrange("(kc p) m -> p kc m", p=128)
                    for q4 in range(4):
                        eng.dma_start(wdst[:, q4 * 4:(q4 + 1) * 4, :],
                                      wr[:, q4 * 4:(q4 + 1) * 4, :])

                def ep_qk(kind, h, ps, rt, s0):
                    dst = (q_sb if kind == "q" else k_sb)[h]
                    sq = p1.tile([128, SCH], DT16, tag="sqv", bufs=3,
                                 name="sq")
                    nc.scalar.square(sq[:], ps[:])
                    ssum = p1ps.tile([128, SCH], F32, tag="ssum", bufs=1,
                                     name="ssum")
                    nc.tensor.matmul(ssum[:], ones_sq[:], sq[:],
                                     start=True, stop=True)
                    # 1/sqrt(var+eps) = exp(-0.5*ln(var+eps)); stays on
                    # ScalarE, avoids the ~2us DVE reciprocal in the chain
                    lnv = p1.tile([128, SCH], F32, tag="sqv", bufs=3,
                                  name="lnv")
                    nc.scalar.activation(
                        lnv[:], ssum[:], mybir.ActivationFunctionType.Ln,
                        bias=eps_t[:], scale=1.0 / HD)
                    rstd = p1.tile([128, SCH], DT16, tag="sqv", bufs=3,
                                   name="rstd")
                    nc.scalar.activation(
                        rstd[:], lnv[:], mybir.ActivationFunctionType.Exp,
                        scale=-0.5)
                    qn = p1.tile([128, SCH], DT16, tag="qn", bufs=2,
                                 name="qn")
                    nc.vector.tensor_mul(qn[:], ps[:], rstd[:])
                    xr, xi = qn[0:64, :], qn[64:128, :]
                    ta = p1.tile([64, SCH], DT16, tag="rot0", bufs=2,
                                 name="ta")
                    tb = p1.tile([64, SCH], DT16, tag="rot1", bufs=2,
                                 name="tb")
                    nc.vector.tensor_mul(ta[:], xr, rt[0:64, 0, :])
                    nc.vector.tensor_mul(tb[:], xi, rt[64:128, 0, :])
                    nc.vector.tensor_sub(dst[0:64, s0:s0 + SCH],
                                         ta[:], tb[:])
                    tc2 = p1.tile([64, SCH], DT16, tag="rot0", bufs=2,
                                  name="tc2")
                    td = p1.tile([64, SCH], DT16, tag="rot1", bufs=2,
                                  name="td")
                    nc.vector.tensor_mul(tc2[:], xr, rt[0:64, 1, :])
                    nc.vector.tensor_mul(td[:], xi, rt[64:128, 1, :])
                    nc.vector.tensor_add(dst[64:128, s0:s0 + SCH],
                                         tc2[:], td[:])

                pend_vtr = []

                def flush_vtr():
                    # v transposes deferred one epilogue further so the PE
                    # never waits on the DVE vt copy
                    while pend_vtr:
                        vt, h, tile0 = pend_vtr.pop(0)
                        for st in range(4):
                            tp = p1ps.tile([128, 128], DT16, tag="vtp",
                                           bufs=2, name="tp")
                            nc.tensor.transpose(
                                tp[:], vt[:, st * 128:(st + 1) * 128],
                                ident[:])
                            nc.vector.tensor_copy(
                                v_sb[h][:, tile0 + st, 0:HD], tp[:])

                def ep_v(h, ps, tile0):
                    vt = p1.tile([128, SCH], DT16, tag="vt", bufs=2,
                                 name="vt")
                    nc.vector.tensor_copy(vt[:], ps[:])
                    pend_vtr.append((vt, h, tile0))

                def p1_epilogue(kind, h, ps, rt, sc):
                    flush_vtr()
                    if kind == "v":
                        ep_v(h, ps, sc * 4)
                    else:
                        ep_qk(kind, h, ps, rt, sc * SCH)

                # 2-deep software pipeline: each output's epilogue is
                # emitted two matmul groups later, giving ScalarE/DVE a
                # full group of slack before the PE consumes their output
                pend = []
                xr_ap = xt.ap().rearrange("(kc p) s -> p kc s", p=128)
                rq_ap = rq.ap().rearrange("f p s -> p f s")
                rk_ap = rk.ap().rearrange("f p s -> p f s")
                for sc in range(NSCH):
                    s0 = sc * SCH
                    # split the 2MB x chunk across both HWDGE queues
                    # (SP + Act) and triple-buffer so the PE never waits
                    xt_t = p1.tile([128, KC, SCH], DT16, tag="xt", bufs=3)
                    nc.sync.dma_start(xt_t[:, 0:8, :],
                                      xr_ap[:, 0:8, s0:s0 + SCH])
                    nc.scalar.dma_start(xt_t[:, 8:16, :],
                                        xr_ap[:, 8:16, s0:s0 + SCH])
                    rq_t = p1.tile([128, 2, SCH], DT16, tag="rq", bufs=2)
                    rk_t = p1.tile([128, 2, SCH], DT16, tag="rk", bufs=2)
                    if sc == 0:
                        # startup: wq right behind chunk 0's x on the two
                        # HWDGE queues (the first matmul groups need only
                        # those); rotary and wk/wv stream in behind,
                        # kc-split across both queues, while the q groups
                        # run (kind-major order below)
                        for wsb_, wsrc in ((wq_s, wqt), (None, None),
                                           (wk_s, wkt), (wv_s, wvt)):
                            if wsb_ is None:
                                nc.sync.dma_start(
                                    rq_t[:], rq_ap[:, :, s0:s0 + SCH])
                                nc.scalar.dma_start(
                                    rk_t[:], rk_ap[:, :, s0:s0 + SCH])
                                continue
                            wr_ = wsrc.ap().rearrange("(kc p) m -> p kc m",
                                                      p=128)
                            nc.sync.dma_start(wsb_[:, 0:8, :],
                                              wr_[:, 0:8, :])
                            nc.scalar.dma_start(wsb_[:, 8:16, :],
                                                wr_[:, 8:16, :])
                    else:
                        nc.sync.dma_start(rq_t[:],
                                          rq_ap[:, :, s0:s0 + SCH])
                        nc.scalar.dma_start(rk_t[:],
                                            rk_ap[:, :, s0:s0 + SCH])

                    if sc == 0 or sc == NSCH - 1:
                        # kind-major: chunk 0 so q groups run while wk/wv
                        # stream in; the last chunk so the final pipeline
                        # flush drains cheap v epilogues, not q/k chains
                        order = [(h, kind) for kind in ("q", "k", "v")
                                 for h in range(HPC)]
                    else:
                        order = [(h, kind) for h in range(HPC)
                                 for kind in ("q", "k", "v")]
                    for h, kind in order:
                        hs = h * HD
                        wsb = {"q": wq_s, "k": wk_s, "v": wv_s}[kind]
                        rt = rq_t if kind == "q" else rk_t
                        ps = p1ps.tile([128, SCH], F32, tag="mm",
                                       bufs=5)
                        for kc in range(KC):
                            nc.tensor.matmul(
                                ps[:], wsb[:, kc, hs:hs + HD],
                                xt_t[:, kc, :],
                                start=(kc == 0), stop=(kc == KC - 1))
                        pend.append((kind, h, ps, rt, sc))
                        if len(pend) > 2:
                            p1_epilogue(*pend.pop(0))
                for item in pend:
                    p1_epilogue(*item)
                flush_vtr()

            # ---------------- Phase 2: attention ----------------
            # 4 passes; pass t computes q-blocks m=2t,2t+1 (512 wide) for
            # every (b,h). Output ownership is stride-4 interleaved so each
            # 512-block contributes one 128-tile to every dest core of its
            # batch; after each pass an AllToAll ships a [2048,256] chunk
            # and the previous pass's output projection runs behind it.
            # last pass ships its two 128-col halves separately so the
            # final AllToAll overlaps the last blocks' compute
            a2a_in_t = [dramp.tile([D, 256], DT16, name=f"a2a_in{t}")
                        for t in range(NT - 1)]
            a2a_out_t = [dramp.tile([D, 256], DT16, name=f"a2a_out{t}")
                         for t in range(NT - 1)]
            a2a_in_h = [dramp.tile([D, 128], DT16, name=f"a2a_inh{u}")
                        for u in range(2)]
            a2a_out_h = [dramp.tile([D, 128], DT16, name=f"a2a_outh{u}")
                         for u in range(2)]
            with tc.tile_pool(name="p2", bufs=1) as p2, \
                 tc.tile_pool(name="p3", bufs=1) as p3, \
                 tc.tile_pool(name="p2ps", bufs=1,
                              space=bass.MemorySpace.PSUM) as p2ps:
                wo_ap = wot.ap().rearrange("(kc p) m -> p kc m", p=128)
                wo_tiles = {}

                def emit_wo_dma(t, n):
                    wt = p3.tile([128, KC, 512], DT16, tag="wo", bufs=3,
                                 name="wo_t")
                    eng = nc.sync if n % 2 == 0 else nc.scalar
                    eng.dma_start(wt[:],
                                  wo_ap[:, :, n * 512:(n + 1) * 512])
                    wo_tiles[(t, n)] = wt

                pend_tr = []

                def flush_tr():
                    # PE part of the previous block's epilogue (transposes
                    # + staging copies), deferred one block so the DVE/Act
                    # normalization chain never stalls the PE
                    while pend_tr:
                        att, att_c, b, h, sub, mpar = pend_tr.pop(0)
                        tp2 = p2ps.tile([128, 128], DT16, tag="aux", bufs=2,
                                        name="tp2")
                        nc.tensor.transpose(tp2[:], att[:], ident[:])
                        nc.vector.tensor_copy(
                            att_c[:, b, h, sub, mpar, :], tp2[:])

                def attention_block512(b, h, m, att_c, mpar):
                    qc = b * S + m * 512
                    # 4 PV accumulators packed 2 per PSUM bank
                    ops = p2ps.tile([128, 4, 256], F32, tag="ops", bufs=1,
                                    name="ops")
                    def emit_pv(pb, kq2):
                        for i in range(2):
                            jt = b * 32 + kq2 * 2 + i
                            for sub in range(4):
                                # start=True clears has_written for the
                                # WHOLE bank, so with 2 accumulators per
                                # bank only the bank-leading sub (0, 2) of
                                # the very first matmul may set it; the
                                # other accumulators' first write lands on
                                # cleared bits and overwrites stale data.
                                nc.tensor.matmul(
                                    ops[:, sub, 0:HD + 1],
                                    pb[:, i, sub * 128:(sub + 1) * 128],
                                    v_sb[h][:, jt, :],
                                    start=(kq2 == 0 and i == 0
                                           and sub % 2 == 0),
                                    stop=(kq2 == 15 and i == 1),
                                    skip_group_check=True)

                    # 2-deep pipeline: PV for pair kq2-2 emitted after
                    # QK/exp of kq2 so ScalarE has slack before PE consumes
                    pending = []
                    for kq2 in range(16):
                        scs = p2ps.tile([128, 2, 512], F32, tag="scs",
                                        bufs=2, name="scs")
                        for i in range(2):
                            kc0 = b * S + (kq2 * 2 + i) * 128
                            nc.tensor.matmul(
                                scs[:, i, :],
                                k_sb[h][:, kc0:kc0 + 128],
                                q_sb[h][:, qc:qc + 512],
                                start=True, stop=True)
                        pb = p2.tile([128, 2, 512], BF16, tag="pb", bufs=5,
                                     name="pb")
                        nc.scalar.activation(
                            pb[:], scs[:],
                            mybir.ActivationFunctionType.Exp, scale=ISQ)
                        pending.append((pb, kq2))
                        if len(pending) > 3:
                            emit_pv(*pending.pop(0))
                        if kq2 == 6:
                            flush_tr()
                    for item in pending:
                        emit_pv(*item)
                    # epilogue: row sums sit at [:, sub, 128]; reciprocal
                    # on DVE straight out of PSUM (ScalarE is the binding
                    # engine in this phase; the deferred transposes absorb
                    # the chain latency)
                    rs = p2.tile([128, 4], F32, tag="rs", bufs=2, name="rs")
                    nc.vector.reciprocal(rs[:], ops[:, :, HD:HD + 1])
                    for sub in range(4):
                        att = p2.tile([128, 128], DT16, tag="att", bufs=8,
                                      name="att")
                        nc.vector.tensor_scalar_mul(
                            att[:], ops[:, sub, 0:HD], rs[:, sub:sub + 1])
                        pend_tr.append((att, att_c, b, h, sub, mpar))

                at_s_tiles = {}

                def load_at_s(t):
                    # prefetch the projection input for pass t as soon as
                    # its AllToAll result can land (emitted early in pass
                    # t+1 so the sync queue isn't clogged by the scatter)
                    at_s = p3.tile([128, KC, 256], DT16, tag="at_s",
                                   bufs=2, name="at_s")
                    nc.sync.dma_start(
                        at_s[:],
                        a2a_out_t[t][:].rearrange(
                            "(kc p) s -> p kc s", p=128))
                    at_s_tiles[t] = (at_s,)

                def load_at_s_half(t, u, eng):
                    ah = p3.tile([128, KC, 128], DT16, tag="at_sh",
                                 bufs=2, name="at_sh")
                    eng.dma_start(
                        ah[:],
                        a2a_out_h[u][:].rearrange(
                            "(kc p) s -> p kc s", p=128))
                    at_s_tiles.setdefault(t, []).append(ah)

                def proj_piece(t, n, st, ats, wt):
                    row0 = t * 256 + st * 128
                    po = p2ps.tile([128, 512], F32, tag="aux",
                                   bufs=2, name="po")
                    for kc in range(KC):
                        if len(ats) == 1:
                            lhs = ats[0][:, kc, st * 128:(st + 1) * 128]
                        else:
                            lhs = ats[st][:, kc, :]
                        nc.tensor.matmul(
                            po[:], lhs, wt[:, kc, :],
                            start=(kc == 0), stop=(kc == KC - 1))
                    ob = p3.tile([128, 512], F32, tag="ob", bufs=2,
                                 name="ob")
                    nc.vector.tensor_copy(ob[:], po[:])
                    nc.sync.dma_start(
                        out_ext.ap()[row0:row0 + 128,
                                     n * 512:(n + 1) * 512], ob[:])

                def proj_run(t, nlist, fire=()):
                    # fire: wo-piece DMAs to emit now; their slot waits
                    # resolve via this proj's own PE progress, so no
                    # engine head-of-line blocks
                    for key in fire:
                        if key[0] < NT and key not in wo_tiles:
                            emit_wo_dma(*key)
                    ats = at_s_tiles[t]
                    for n in nlist:
                        wt = wo_tiles.pop((t, n))
                        for st in range(2):
                            proj_piece(t, n, st, ats, wt)
                    if nlist[-1] == 3:
                        at_s_tiles.pop(t)

                def proj_run_final(t):
                    # interleaved piece order: all but the st=1 tail run
                    # before the last AllToAll lands
                    emit_wo_dma(t, 2)
                    ats = at_s_tiles[t]
                    wts = {n: wo_tiles.pop((t, n)) for n in (0, 1)}
                    order = [(0, 0), (1, 0), (0, 1), (2, 0), (1, 1),
                             (3, 0), (2, 1), (3, 1)]
                    for i, (n, st) in enumerate(order):
                        if i == 2:
                            emit_wo_dma(t, 3)
                        if n not in wts:
                            wts[n] = wo_tiles.pop((t, n))
                        proj_piece(t, n, st, ats, wts[n])
                    at_s_tiles.pop(t)

                def scatter_a2a(t, att_c, mpar=None):
                    # dest core j rows <- its q-tiles, both heads
                    for j in range(NCORES):
                        bb, g = j // 4, j % 4
                        for h in range(HPC):
                            r0 = j * HPC * HD + h * HD
                            if mpar is None:
                                nc.sync.dma_start(
                                    a2a_in_t[t][r0:r0 + HD, :],
                                    att_c[:, bb, h, g, :, :])
                            else:
                                nc.sync.dma_start(
                                    a2a_in_h[mpar][r0:r0 + HD, :],
                                    att_c[:, bb, h, g, mpar, :])
                    if mpar is None:
                        ins, outs = a2a_in_t[t], a2a_out_t[t]
                    else:
                        ins, outs = a2a_in_h[mpar], a2a_out_h[mpar]
                    nc.gpsimd.collective_compute(
                        "AllToAll", mybir.AluOpType.bypass,
                        ins=[ins[:].opt()], outs=[outs[:].opt()],
                        replica_groups=[list(range(NCORES))])

                for t in range(NT):
                    nblk = 0
                    last = t == NT - 1
                    for mpar_o in ((0, 1),) if not last else ((0,), (1,)):
                        att_c = p2.tile([128, B, HPC, 4, 2, 128], DT16,
                                        tag="attc", bufs=2, name="att_c")
                        for b in range(B):
                            for h in range(HPC):
                                for mpar in mpar_o:
                                    attention_block512(b, h, 2 * t + mpar,
                                                       att_c, mpar)
                                    nblk += 1
                                    if t == 0 and nblk == 5:
                                        emit_wo_dma(0, 0)
                                    elif t == 0 and nblk == 7:
                                        emit_wo_dma(0, 1)
                                    elif t > 0 and nblk == 1:
                                        load_at_s(t - 1)
                        # a slice of the previous pass's projection before
                        # the epilogue flush: PE work that covers the last
                        # block's DVE normalization latency
                        if not last:
                            if t > 0:
                                proj_run(t - 1, [0],
                                         fire=((t - 1, 2), (t - 1, 3)))
                            flush_tr()
                            scatter_a2a(t, att_c)
                        elif mpar_o[0] == 0:
                            proj_run(t - 1, [0],
                                     fire=((t - 1, 2), (t - 1, 3)))
                            flush_tr()
                            scatter_a2a(t, att_c, mpar=0)
                            load_at_s_half(t, 0, nc.sync)
                            proj_run(t - 1, [1])
                        else:
                            # second half: its AllToAll is tail-critical,
                            # scatter first, then the rest of proj(t-1)
                            flush_tr()
                            scatter_a2a(t, att_c, mpar=1)
                            proj_run(t - 1, [2, 3],
                                     fire=((t, 0), (t, 1)))
                            load_at_s_half(t, 1, nc.sync)
                    if 0 < t < NT - 1:
                        proj_run(t - 1, [1, 2, 3], fire=((t, 0), (t, 1)))
                proj_run(NT - 1, [0, 1, 2, 3],
                         fire=((NT - 1, 2), (NT - 1, 3)))
            qkvp.release()

    nc.compile()
    return nc


def _prep(inputs):
    x = np.asarray(inputs["x"], np.float32)
    freqs = np.asarray(inputs["freqs_cis"], np.float32)
    wq = np.asarray(inputs["wq"], np.float32)
    wk = np.asarray(inputs["wk"], np.float32)
    wv = np.asarray(inputs["wv"], np.float32)
    wo = np.asarray(inputs["wo"], np.float32)
    nqw = np.asarray(inputs["norm_q_w"], np.float32)
    nkw = np.asarray(inputs["norm_k_w"], np.float32)

    bf = np.float16
    xt = np.ascontiguousarray(x.reshape(BS, D).T).astype(bf)

    # de-interleave rotary pairs within each head's 128 rows
    perm = np.concatenate([np.arange(0, HD, 2), np.arange(1, HD, 2)])
    full_perm = (np.arange(H)[:, None] * HD + perm[None, :]).reshape(-1)
    wq_p = wq[full_perm]
    wk_p = wk[full_perm]

    fr = freqs.reshape(BS, HD)
    cos = np.ascontiguousarray(fr[:, :64].T)   # [64, BS]
    sin = np.ascontiguousarray(fr[:, 64:].T)
    def rot_coefs(w):
        wr = w[0::2][:, None]
        wi = w[1::2][:, None]
        plane0 = np.concatenate([wr * cos, wi * sin], axis=0)   # [128, BS]
        plane1 = np.concatenate([wr * sin, wi * cos], axis=0)
        return np.stack([plane0, plane1]).astype(bf)
    rq = rot_coefs(nqw)
    rk = rot_coefs(nkw)

    wot = np.ascontiguousarray(wo.T).astype(bf)

    in_maps = []
    for c in range(NCORES):
        r0, r1 = c * HPC * HD, (c + 1) * HPC * HD
        in_maps.append({
            "xt": xt,
            "wqt": np.ascontiguousarray(wq_p[r0:r1].T).astype(bf),
            "wkt": np.ascontiguousarray(wk_p[r0:r1].T).astype(bf),
            "wvt": np.ascontiguousarray(wv[r0:r1].T).astype(bf),
            "wot": wot,
            "rq": rq,
            "rk": rk,
        })
    return in_maps


def kernel(**inputs):
    if "nc" not in _CACHE:
        _CACHE["nc"] = _build()
    nc = _CACHE["nc"]
    in_maps = _prep(inputs)
    res = run_bass_kernel_spmd(nc, in_maps, list(range(NCORES)),
                               **_CACHE.get("run_kwargs", {}))
    _CACHE["last_result"] = res
    # core j=b*4+g owns q-tiles {4k+g, k=0..7} of batch b (row block k)
    out = np.empty((B, S, D), np.float32)
    for j in range(NCORES):
        bb, g = j // 4, j % 4
        rj = np.asarray(res.results[j]["out"]).reshape(8, 128, D)
        for k in range(8):
            t0 = (4 * k + g) * 128
            out[bb, t0:t0 + 128, :] = rj[k]
    return out


# revision 46
# speedup vs baseline: 1.0032x; 1.0032x over previous
"""Distributed Trainium2 kernel for nn_Attention (B=2,S=4096,D=2048,H=16).

Tensor-parallel over heads across 8 NeuronCores; core c owns heads 2c,2c+1.

Host prep (free): x -> xT [D, B*S] fp16; per-core wq/wk/wv column slices
pre-transposed, with rotary pair de-interleave folded into the wq/wk row
permutation; rotary cos/sin combined with the RMS-norm weights into 4
coefficient planes; wo pre-transposed.

Per core:
  1. QKV over 512-wide s-chunks (x DMA split across both HWDGE queues,
     triple-buffered, so the PE never waits and HAM stays warm). RMS-norm
     partition-reduction via a ones matmul into a [128,512] broadcast
     layout; 1/sqrt(var) computed as exp(-0.5*ln(var)) on ScalarE (keeps
     the slow DVE reciprocal off the dependency chain); rotary on VectorE;
     v PE-transposed to natural [s, hd+1] layout with an appended ones
     column (copies on DVE). Epilogues software-pipelined one matmul group
     behind.
  2. Attention per (b, head, 512-wide q block): scoresT = kT.T @ qT with
     N=512 matmuls (LDWEIGHTS fully hidden), exp on ScalarE on [128,2,512]
     score pairs straight out of PSUM (bf16 out; scores bounded so no
     max-subtraction), PV accumulates probsT.T @ [v|1] into a [128,4,256]
     PSUM tile (two 129-wide accumulators per bank) giving attention output
     and softmax row sums in one pass. PV emitted 2 exp-pairs behind QK.
  3. Output ownership is stride-4 interleaved: core j owns q-tiles
     {4k + j%4} of batch j//4. Each of 4 passes computes q-blocks 2t,2t+1
     for every (b,h), AllToAll's one 256-col chunk per dest, and the output
     projection for the previous pass runs behind it with wo streamed from
     DRAM in 2MB pieces (no bulk 8MB stall).
Host reassembles the interleaved row blocks.
"""
import sys

sys.path.insert(0, "/opt/trn_rl_repo")

import numpy as np
import ml_dtypes

import concourse.bass as bass
import concourse.bacc as bacc
import concourse.mybir as mybir
import concourse.tile as tile
from concourse import masks
from concourse.bass_utils import run_bass_kernel_spmd

DT16 = mybir.dt.float16
BF16 = mybir.dt.bfloat16
F32 = mybir.dt.float32

B, S, D, H = 2, 4096, 2048, 16
HD = 128                  # head dim
NCORES = 8
HPC = H // NCORES         # heads per core = 2
BS = B * S                # 8192
KC = D // 128             # 16 contraction chunks
SCH = 512                 # s-chunk for QKV phase
NSCH = BS // SCH          # 16
SLICE = BS // NCORES      # 1024 output rows per core
NT = 4                    # attention/a2a passes
EPS = 1e-5
ISQ = 1.0 / np.sqrt(HD)

_CACHE = {}


def _build():
    nc = bacc.Bacc("TRN2", target_bir_lowering=False, debug=False,
                   num_devices=NCORES)

    xt = nc.dram_tensor("xt", [D, BS], DT16, kind="ExternalInput")
    wqt = nc.dram_tensor("wqt", [D, HPC * HD], DT16, kind="ExternalInput")
    wkt = nc.dram_tensor("wkt", [D, HPC * HD], DT16, kind="ExternalInput")
    wvt = nc.dram_tensor("wvt", [D, HPC * HD], DT16, kind="ExternalInput")
    wot = nc.dram_tensor("wot", [D, D], DT16, kind="ExternalInput")
    # plane 0 rows = [A(64); B(64)], plane 1 rows = [C(64); D(64)] so every
    # rotary multiply pairs SBUF operands with equal base partition.
    rq = nc.dram_tensor("rq", [2, 128, BS], DT16, kind="ExternalInput")
    rk = nc.dram_tensor("rk", [2, 128, BS], DT16, kind="ExternalInput")
    out_ext = nc.dram_tensor("out", [SLICE, D], F32, kind="ExternalOutput")

    with tile.TileContext(nc) as tc:
        with tc.tile_pool(name="persist", bufs=1) as pp, \
             tc.tile_pool(name="dramp", bufs=1, space="DRAM") as dramp:
            ident = pp.tile([128, 128], DT16)
            masks.make_identity(nc, ident[:])
            ones_sq = pp.tile([128, 128], DT16)
            nc.gpsimd.memset(ones_sq[:], 1.0)
            eps_t = pp.tile([128, 1], F32)
            nc.gpsimd.memset(eps_t[:], EPS)

            # per-head tensors living through phases 1-2
            qkvp = tc.alloc_tile_pool(name="qkvp", bufs=1)
            q_sb = [qkvp.tile([128, BS], DT16, name=f"q{h}")
                    for h in range(HPC)]
            k_sb = [qkvp.tile([128, BS], DT16, name=f"k{h}")
                    for h in range(HPC)]
            # v in natural layout per 128-row s-tile, ones column at 128
            v_sb = [qkvp.tile([128, BS // 128, HD + 1], BF16, name=f"v{h}")
                    for h in range(HPC)]
            for h in range(HPC):
                nc.gpsimd.memset(v_sb[h][:, :, HD:HD + 1], 1.0)

            # ---------------- Phase 1: QKV + RMS + rotary ----------------
            with tc.tile_pool(name="p1", bufs=1) as p1, \
                 tc.tile_pool(name="p1ps", bufs=1,
                              space=bass.MemorySpace.PSUM) as p1ps:
                wq_s = p1.tile([128, KC, HPC * HD], DT16)
                wk_s = p1.tile([128, KC, HPC * HD], DT16)
                wv_s = p1.tile([128, KC, HPC * HD], DT16)

                def ep_qk(kind, h, ps, rt, s0):
                    dst = (q_sb if kind == "q" else k_sb)[h]
                    sq = p1.tile([128, SCH], DT16, tag="sqv", bufs=3,
                                 name="sq")
                    nc.scalar.square(sq[:], ps[:])
                    ssum = p1ps.tile([128, SCH], F32, tag="ssum", bufs=1,
                                     name="ssum")
                    nc.tensor.matmul(ssum[:], ones_sq[:], sq[:],
                                     start=True, stop=True)
                    # 1/sqrt(var+eps) = exp(-0.5*ln(var+eps)); stays on
                    # ScalarE, avoids the ~2us DVE reciprocal in the chain
                    lnv = p1.tile([128, SCH], F32, tag="sqv", bufs=3,
                                  name="lnv")
                    nc.scalar.activation(
                        lnv[:], ssum[:], mybir.ActivationFunctionType.Ln,
                        bias=eps_t[:], scale=1.0 / HD)
                    rstd = p1.tile([128, SCH], DT16, tag="sqv", bufs=3,
                                   name="rstd")
                    nc.scalar.activation(
                        rstd[:], lnv[:], mybir.ActivationFunctionType.Exp,
                        scale=-0.5)
                    qn = p1.tile([128, SCH], DT16, tag="qn", bufs=2,
                                 name="qn")
                    nc.vector.tensor_mul(qn[:], ps[:], rstd[:])
                    xr, xi = qn[0:64, :], qn[64:128, :]
                    ta = p1.tile([64, SCH], DT16, tag="rot0", bufs=2,
                                 name="ta")
                    tb = p1.tile([64, SCH], DT16, tag="rot1", bufs=2,
                                 name="tb")
                    nc.vector.tensor_mul(ta[:], xr, rt[0:64, 0, :])
                    nc.vector.tensor_mul(tb[:], xi, rt[64:128, 0, :])
                    nc.vector.tensor_sub(dst[0:64, s0:s0 + SCH],
                                         ta[:], tb[:])
                    tc2 = p1.tile([64, SCH], DT16, tag="rot0", bufs=2,
                                  name="tc2")
                    td = p1.tile([64, SCH], DT16, tag="rot1", bufs=2,
                                  name="td")
                    nc.vector.tensor_mul(tc2[:], xr, rt[0:64, 1, :])
                    nc.vector.tensor_mul(td[:], xi, rt[64:128, 1, :])
                    nc.vector.tensor_add(dst[64:128, s0:s0 + SCH],
                                         tc2[:], td[:])

                pend_vtr = []

                def flush_vtr():
                    # v transposes deferred one epilogue further so the PE
                    # never waits on the DVE vt copy
                    while pend_vtr:
                        vt, h, tile0 = pend_vtr.pop(0)
                        for st in range(4):
                            tp = p1ps.tile([128, 128], DT16, tag="vtp",
                                           bufs=2, name="tp")
                            nc.tensor.transpose(
                                tp[:], vt[:, st * 128:(st + 1) * 128],
                                ident[:])
                            nc.vector.tensor_copy(
                                v_sb[h][:, tile0 + st, 0:HD], tp[:])

                def ep_v(h, ps, tile0):
                    vt = p1.tile([128, SCH], DT16, tag="vt", bufs=2,
                                 name="vt")
                    nc.vector.tensor_copy(vt[:], ps[:])
                    pend_vtr.append((vt, h, tile0))

                def p1_epilogue(kind, h, ps, rt, sc):
                    flush_vtr()
                    if kind == "v":
                        ep_v(h, ps, sc * 4)
                    else:
                        ep_qk(kind, h, ps, rt, sc * SCH)

                # 2-deep software pipeline: each output's epilogue is
                # emitted two matmul groups later, giving ScalarE/DVE a
                # full group of slack before the PE consumes their output
                pend = []
                xr_ap = xt.ap().rearrange("(kc p) s -> p kc s", p=128)
                rq_ap = rq.ap().rearrange("f p s -> p f s")
                rk_ap = rk.ap().rearrange("f p s -> p f s")
                for sc in range(NSCH):
                    s0 = sc * SCH
                    # split the 2MB x chunk across both HWDGE queues
                    # (SP + Act) and triple-buffer so the PE never waits
                    xt_t = p1.tile([128, KC, SCH], DT16, tag="xt", bufs=3)
                    nc.sync.dma_start(xt_t[:, 0:8, :],
                                      xr_ap[:, 0:8, s0:s0 + SCH])
                    nc.scalar.dma_start(xt_t[:, 8:16, :],
                                        xr_ap[:, 8:16, s0:s0 + SCH])
                    rq_t = p1.tile([128, 2, SCH], DT16, tag="rq", bufs=2)
                    rk_t = p1.tile([128, 2, SCH], DT16, tag="rk", bufs=2)
                    if sc == 0:
                        # startup: wq right behind chunk 0's x on the two
                        # HWDGE queues (the first matmul groups need only
                        # those); rotary and wk/wv stream in behind,
                        # kc-split across both queues, while the q groups
                        # run (kind-major order below)
                        for wsb_, wsrc in ((wq_s, wqt), (None, None),
                                           (wk_s, wkt), (wv_s, wvt)):
                            if wsb_ is None:
                                nc.sync.dma_start(
                                    rq_t[:], rq_ap[:, :, s0:s0 + SCH])
                                nc.scalar.dma_start(
                                    rk_t[:], rk_ap[:, :, s0:s0 + SCH])
                                continue
                            wr_ = wsrc.ap().rearrange("(kc p) m -> p kc m",
                                                      p=128)
                            nc.sync.dma_start(wsb_[:, 0:8, :],
                                              wr_[:, 0:8, :])
                            nc.scalar.dma_start(wsb_[:, 8:16, :],
                                                wr_[:, 8:16, :])
                    else:
                        nc.sync.dma_start(rq_t[:],
                                          rq_ap[:, :, s0:s0 + SCH])
                        nc.scalar.dma_start(rk_t[:],
                                            rk_ap[:, :, s0:s0 + SCH])

                    if sc == 0 or sc == NSCH - 1:
                        # kind-major: chunk 0 so q groups run while wk/wv
                        # stream in; the last chunk so the final pipeline
                        # flush drains cheap v epilogues, not q/k chains
                        order = [(h, kind) for kind in ("q", "k", "v")
                                 for h in range(HPC)]
                    else:
                        order = [(h, kind) for h in range(HPC)
                                 for kind in ("q", "k", "v")]
                    for h, kind in order:
                        hs = h * HD
                        wsb = {"q": wq_s, "k": wk_s, "v": wv_s}[kind]
                        rt = rq_t if kind == "q" else rk_t
                        ps = p1ps.tile([128, SCH], F32, tag="mm",
                                       bufs=5)
                        for kc in range(KC):
                            nc.tensor.matmul(
                                ps[:], wsb[:, kc, hs:hs + HD],
                                xt_t[:, kc, :],
                                start=(kc == 0), stop=(kc == KC - 1))
                        pend.append((kind, h, ps, rt, sc))
                        if len(pend) > 2:
                            p1_epilogue(*pend.pop(0))
                for item in pend:
                    p1_epilogue(*item)
                flush_vtr()

            # ---------------- Phase 2: attention ----------------
            # 4 passes; pass t computes q-blocks m=2t,2t+1 (512 wide) for
            # every (b,h). Output ownership is stride-4 interleaved so each
            # 512-block contributes one 128-tile to every dest core of its
            # batch; after each pass an AllToAll ships a [2048,256] chunk
            # and the previous pass's output projection runs behind it.
            # last pass ships its two 128-col halves separately so the
            # final AllToAll overlaps the last blocks' compute
            a2a_in_t = [dramp.tile([D, 256], DT16, name=f"a2a_in{t}")
                        for t in range(NT - 1)]
            a2a_out_t = [dramp.tile([D, 256], DT16, name=f"a2a_out{t}")
                         for t in range(NT - 1)]
            a2a_in_h = [dramp.tile([D, 128], DT16, name=f"a2a_inh{u}")
                        for u in range(2)]
            a2a_out_h = [dramp.tile([D, 128], DT16, name=f"a2a_outh{u}")
                         for u in range(2)]
            with tc.tile_pool(name="p2", bufs=1) as p2, \
                 tc.tile_pool(name="p3", bufs=1) as p3, \
                 tc.tile_pool(name="p2ps", bufs=1,
                              space=bass.MemorySpace.PSUM) as p2ps:
                wo_ap = wot.ap().rearrange("(kc p) m -> p kc m", p=128)
                wo_tiles = {}

                def emit_wo_dma(t, n):
                    wt = p3.tile([128, KC, 512], DT16, tag="wo", bufs=3,
                                 name="wo_t")
                    eng = nc.sync if n % 2 == 0 else nc.scalar
                    eng.dma_start(wt[:],
                                  wo_ap[:, :, n * 512:(n + 1) * 512])
                    wo_tiles[(t, n)] = wt

                pend_tr = []

                def flush_tr():
                    # PE part of the previous block's epilogue (transposes
                    # + staging copies), deferred one block so the DVE/Act
                    # normalization chain never stalls the PE
                    while pend_tr:
                        att, att_c, b, h, sub, mpar = pend_tr.pop(0)
                        tp2 = p2ps.tile([128, 128], DT16, tag="aux", bufs=2,
                                        name="tp2")
                        nc.tensor.transpose(tp2[:], att[:], ident[:])
                        nc.vector.tensor_copy(
                            att_c[:, b, h, sub, mpar, :], tp2[:])

                def attention_block512(b, h, m, att_c, mpar):
                    qc = b * S + m * 512
                    # 4 PV accumulators packed 2 per PSUM bank
                    ops = p2ps.tile([128, 4, 256], F32, tag="ops", bufs=1,
                                    name="ops")
                    def emit_pv(pb, kq2):
                        for i in range(2):
                            jt = b * 32 + kq2 * 2 + i
                            for sub in range(4):
                                # start=True clears has_written for the
                                # WHOLE bank, so with 2 accumulators per
                                # bank only the bank-leading sub (0, 2) of
                                # the very first matmul may set it; the
                                # other accumulators' first write lands on
                                # cleared bits and overwrites stale data.
                                nc.tensor.matmul(
                                    ops[:, sub, 0:HD + 1],
                                    pb[:, i, sub * 128:(sub + 1) * 128],
                                    v_sb[h][:, jt, :],
                                    start=(kq2 == 0 and i == 0
                                           and sub % 2 == 0),
                                    stop=(kq2 == 15 and i == 1),
                                    skip_group_check=True)

                    # 2-deep pipeline: PV for pair kq2-2 emitted after
                    # QK/exp of kq2 so ScalarE has slack before PE consumes
                    pending = []
                    for kq2 in range(16):
                        scs = p2ps.tile([128, 2, 512], F32, tag="scs",
                                        bufs=2, name="scs")
                        for i in range(2):
                            kc0 = b * S + (kq2 * 2 + i) * 128
                            nc.tensor.matmul(
                                scs[:, i, :],
                                k_sb[h][:, kc0:kc0 + 128],
                                q_sb[h][:, qc:qc + 512],
                                start=True, stop=True)
                        pb = p2.tile([128, 2, 512], BF16, tag="pb", bufs=4,
                                     name="pb")
                        nc.scalar.activation(
                            pb[:], scs[:],
                            mybir.ActivationFunctionType.Exp, scale=ISQ)
                        pending.append((pb, kq2))
                        if len(pending) > 2:
                            emit_pv(*pending.pop(0))
                        if kq2 == 6:
                            flush_tr()
                    for item in pending:
                        emit_pv(*item)
                    # epilogue: row sums sit at [:, sub, 128]; reciprocal
                    # on DVE straight out of PSUM (ScalarE is the binding
                    # engine in this phase; the deferred transposes absorb
                    # the chain latency)
                    rs = p2.tile([128, 4], F32, tag="rs", bufs=2, name="rs")
                    nc.vector.reciprocal(rs[:], ops[:, :, HD:HD + 1])
                    for sub in range(4):
                        att = p2.tile([128, 128], DT16, tag="att", bufs=8,
                                      name="att")
                        nc.vector.tensor_scalar_mul(
                            att[:], ops[:, sub, 0:HD], rs[:, sub:sub + 1])
                        pend_tr.append((att, att_c, b, h, sub, mpar))

                at_s_tiles = {}

                def load_at_s(t):
                    # prefetch the projection input for pass t as soon as
                    # its AllToAll result can land (emitted early in pass
                    # t+1 so the sync queue isn't clogged by the scatter)
                    at_s = p3.tile([128, KC, 256], DT16, tag="at_s",
                                   bufs=2, name="at_s")
                    nc.sync.dma_start(
                        at_s[:],
                        a2a_out_t[t][:].rearrange(
                            "(kc p) s -> p kc s", p=128))
                    at_s_tiles[t] = (at_s,)

                def load_at_s_half(t, u, eng):
                    ah = p3.tile([128, KC, 128], DT16, tag="at_sh",
                                 bufs=2, name="at_sh")
                    eng.dma_start(
                        ah[:],
                        a2a_out_h[u][:].rearrange(
                            "(kc p) s -> p kc s", p=128))
                    at_s_tiles.setdefault(t, []).append(ah)

                def proj_piece(t, n, st, ats, wt):
                    row0 = t * 256 + st * 128
                    po = p2ps.tile([128, 512], F32, tag="aux",
                                   bufs=2, name="po")
                    for kc in range(KC):
                        if len(ats) == 1:
                            lhs = ats[0][:, kc, st * 128:(st + 1) * 128]
                        else:
                            lhs = ats[st][:, kc, :]
                        nc.tensor.matmul(
                            po[:], lhs, wt[:, kc, :],
                            start=(kc == 0), stop=(kc == KC - 1))
                    ob = p3.tile([128, 512], F32, tag="ob", bufs=2,
                                 name="ob")
                    nc.vector.tensor_copy(ob[:], po[:])
                    nc.sync.dma_start(
                        out_ext.ap()[row0:row0 + 128,
                                     n * 512:(n + 1) * 512], ob[:])

                def proj_run(t, nlist, fire=()):
                    # fire: wo-piece DMAs to emit now; their slot waits
                    # resolve via this proj's own PE progress, so no
                    # engine head-of-line blocks
                    for key in fire:
                        if key[0] < NT and key not in wo_tiles:
                            emit_wo_dma(*key)
                    ats = at_s_tiles[t]
                    for n in nlist:
                        wt = wo_tiles.pop((t, n))
                        for st in range(2):
                            proj_piece(t, n, st, ats, wt)
                    if nlist[-1] == 3:
                        at_s_tiles.pop(t)

                def proj_run_final(t):
                    # interleaved piece order: all but the st=1 tail run
                    # before the last AllToAll lands
                    emit_wo_dma(t, 2)
                    ats = at_s_tiles[t]
                    wts = {n: wo_tiles.pop((t, n)) for n in (0, 1)}
                    order = [(0, 0), (1, 0), (0, 1), (2, 0), (1, 1),
                             (3, 0), (2, 1), (3, 1)]
                    for i, (n, st) in enumerate(order):
                        if i == 2:
                            emit_wo_dma(t, 3)
                        if n not in wts:
                            wts[n] = wo_tiles.pop((t, n))
                        proj_piece(t, n, st, ats, wts[n])
                    at_s_tiles.pop(t)

                def scatter_a2a(t, att_c, mpar=None):
                    # dest core j rows <- its q-tiles, both heads
                    for j in range(NCORES):
                        bb, g = j // 4, j % 4
                        for h in range(HPC):
                            r0 = j * HPC * HD + h * HD
                            if mpar is None:
                                nc.sync.dma_start(
                                    a2a_in_t[t][r0:r0 + HD, :],
                                    att_c[:, bb, h, g, :, :])
                            else:
                                nc.sync.dma_start(
                                    a2a_in_h[mpar][r0:r0 + HD, :],
                                    att_c[:, bb, h, g, mpar, :])
                    if mpar is None:
                        ins, outs = a2a_in_t[t], a2a_out_t[t]
                    else:
                        ins, outs = a2a_in_h[mpar], a2a_out_h[mpar]
                    nc.gpsimd.collective_compute(
                        "AllToAll", mybir.AluOpType.bypass,
                        ins=[ins[:].opt()], outs=[outs[:].opt()],
                        replica_groups=[list(range(NCORES))])

                for t in range(NT):
                    nblk = 0
                    last = t == NT - 1
                    for mpar_o in ((0, 1),) if not last else ((0,), (1,)):
                        att_c = p2.tile([128, B, HPC, 4, 2, 128], DT16,
                                        tag="attc", bufs=2, name="att_c")
                        for b in range(B):
                            for h in range(HPC):
                                for mpar in mpar_o:
                                    attention_block512(b, h, 2 * t + mpar,
                                                       att_c, mpar)
                                    nblk += 1
                                    if t == 0 and nblk == 5:
                                        emit_wo_dma(0, 0)
                                    elif t == 0 and nblk == 7:
                                        emit_wo_dma(0, 1)
                                    elif t > 0 and nblk == 1:
                                        load_at_s(t - 1)
                        # a slice of the previous pass's projection before
                        # the epilogue flush: PE work that covers the last
                        # block's DVE normalization latency
                        if not last:
                            if t > 0:
                                proj_run(t - 1, [0],
                                         fire=((t - 1, 2), (t - 1, 3)))
                            flush_tr()
                            scatter_a2a(t, att_c)
                        elif mpar_o[0] == 0:
                            proj_run(t - 1, [0],
                                     fire=((t - 1, 2), (t - 1, 3)))
                            flush_tr()
                            scatter_a2a(t, att_c, mpar=0)
                            load_at_s_half(t, 0, nc.sync)
                            proj_run(t - 1, [1])
                        else:
                            # second half: its AllToAll is tail-critical,
                            # scatter first, then the rest of proj(t-1)
                            flush_tr()
                            scatter_a2a(t, att_c, mpar=1)
                            proj_run(t - 1, [2, 3],
                                     fire=((t, 0), (t, 1)))
                            load_at_s_half(t, 1, nc.sync)
                    if 0 < t < NT - 1:
                        proj_run(t - 1, [1, 2, 3], fire=((t, 0), (t, 1)))
                proj_run(NT - 1, [0, 1, 2, 3],
                         fire=((NT - 1, 2), (NT - 1, 3)))
            qkvp.release()

    nc.compile()
    return nc


def _prep(inputs):
    x = np.asarray(inputs["x"], np.float32)
    freqs = np.asarray(inputs["freqs_cis"], np.float32)
    wq = np.asarray(inputs["wq"], np.float32)
    wk = np.asarray(inputs["wk"], np.float32)
    wv = np.asarray(inputs["wv"], np.float32)
    wo = np.asarray(inputs["wo"], np.float32)
    nqw = np.asarray(inputs["norm_q_w"], np.float32)
    nkw = np.asarray(inputs["norm_k_w"], np.float32)

    bf = np.float16
    xt = np.ascontiguousarray(x.reshape(BS, D).T).astype(bf)

    # de-interleave rotary pairs within each head's 128 rows
    perm = np.concatenate([np.arange(0, HD, 2), np.arange(1, HD, 2)])
    full_perm = (np.arange(H)[:, None] * HD + perm[None, :]).reshape(-1)
    wq_p = wq[full_perm]
    wk_p = wk[full_perm]

    fr = freqs.reshape(BS, HD)
    cos = np.ascontiguousarray(fr[:, :64].T)   # [64, BS]
    sin = np.ascontiguousarray(fr[:, 64:].T)
    def rot_coefs(w):
        wr = w[0::2][:, None]
        wi = w[1::2][:, None]
        plane0 = np.concatenate([wr * cos, wi * sin], axis=0)   # [128, BS]
        plane1 = np.concatenate([wr * sin, wi * cos], axis=0)
        return np.stack([plane0, plane1]).astype(bf)
    rq = rot_coefs(nqw)
    rk = rot_coefs(nkw)

    wot = np.ascontiguousarray(wo.T).astype(bf)

    in_maps = []
    for c in range(NCORES):
        r0, r1 = c * HPC * HD, (c + 1) * HPC * HD
        in_maps.append({
            "xt": xt,
            "wqt": np.ascontiguousarray(wq_p[r0:r1].T).astype(bf),
            "wkt": np.ascontiguousarray(wk_p[r0:r1].T).astype(bf),
            "wvt": np.ascontiguousarray(wv[r0:r1].T).astype(bf),
            "wot": wot,
            "rq": rq,
            "rk": rk,
        })
    return in_maps


def kernel(**inputs):
    if "nc" not in _CACHE:
        _CACHE["nc"] = _build()
    nc = _CACHE["nc"]
    in_maps = _prep(inputs)
    res = run_bass_kernel_spmd(nc, in_maps, list(range(NCORES)),
                               **_CACHE.get("run_kwargs", {}))
    _CACHE["last_result"] = res
    # core j=b*4+g owns q-tiles {4k+g, k=0..7} of batch b (row block k)
    out = np.empty((B, S, D), np.float32)
    for j in range(NCORES):
        bb, g = j // 4, j % 4
        rj = np.asarray(res.results[j]["out"]).reshape(8, 128, D)
        for k in range(8):
            t0 = (4 * k + g) * 128
            out[bb, t0:t0 + 128, :] = rj[k]
    return out


# revision 47
# speedup vs baseline: 1.0048x; 1.0016x over previous
"""Distributed Trainium2 kernel for nn_Attention (B=2,S=4096,D=2048,H=16).

Tensor-parallel over heads across 8 NeuronCores; core c owns heads 2c,2c+1.

Host prep (free): x -> xT [D, B*S] fp16; per-core wq/wk/wv column slices
pre-transposed, with rotary pair de-interleave folded into the wq/wk row
permutation; rotary cos/sin combined with the RMS-norm weights into 4
coefficient planes; wo pre-transposed.

Per core:
  1. QKV over 512-wide s-chunks (x DMA split across both HWDGE queues,
     triple-buffered, so the PE never waits and HAM stays warm). RMS-norm
     partition-reduction via a ones matmul into a [128,512] broadcast
     layout; 1/sqrt(var) computed as exp(-0.5*ln(var)) on ScalarE (keeps
     the slow DVE reciprocal off the dependency chain); rotary on VectorE;
     v PE-transposed to natural [s, hd+1] layout with an appended ones
     column (copies on DVE). Epilogues software-pipelined one matmul group
     behind.
  2. Attention per (b, head, 512-wide q block): scoresT = kT.T @ qT with
     N=512 matmuls (LDWEIGHTS fully hidden), exp on ScalarE on [128,2,512]
     score pairs straight out of PSUM (bf16 out; scores bounded so no
     max-subtraction), PV accumulates probsT.T @ [v|1] into a [128,4,256]
     PSUM tile (two 129-wide accumulators per bank; start=True only on the
     bank-leading accumulator since it clears has_written bank-wide)
     giving attention output and softmax row sums in one pass. PV emitted
     2 exp-pairs behind QK; the epilogue's PE transposes are deferred one
     block so the DVE reciprocal chain never stalls the PE.
  3. Output ownership is stride-4 interleaved: core j owns q-tiles
     {4k + j%4} of batch j//4. Each of 4 passes computes q-blocks 2t,2t+1
     for every (b,h), AllToAll's one 256-col chunk per dest, and the output
     projection for the previous pass runs behind it with wo streamed from
     DRAM in 2MB pieces on alternating queues. The last pass ships its two
     128-col halves separately and interleaves its projection so only the
     st=1 pieces sit behind the final AllToAll's latency.
Host reassembles the interleaved row blocks.
"""
import sys

sys.path.insert(0, "/opt/trn_rl_repo")

import numpy as np
import ml_dtypes

import concourse.bass as bass
import concourse.bacc as bacc
import concourse.mybir as mybir
import concourse.tile as tile
from concourse import masks
from concourse.bass_utils import run_bass_kernel_spmd

DT16 = mybir.dt.float16
BF16 = mybir.dt.bfloat16
F32 = mybir.dt.float32

B, S, D, H = 2, 4096, 2048, 16
HD = 128                  # head dim
NCORES = 8
HPC = H // NCORES         # heads per core = 2
BS = B * S                # 8192
KC = D // 128             # 16 contraction chunks
SCH = 512                 # s-chunk for QKV phase
NSCH = BS // SCH          # 16
SLICE = BS // NCORES      # 1024 output rows per core
NT = 4                    # attention/a2a passes
EPS = 1e-5
ISQ = 1.0 / np.sqrt(HD)

_CACHE = {}


def _build():
    nc = bacc.Bacc("TRN2", target_bir_lowering=False, debug=False,
                   num_devices=NCORES)

    xt = nc.dram_tensor("xt", [D, BS], DT16, kind="ExternalInput")
    wqt = nc.dram_tensor("wqt", [D, HPC * HD], DT16, kind="ExternalInput")
    wkt = nc.dram_tensor("wkt", [D, HPC * HD], DT16, kind="ExternalInput")
    wvt = nc.dram_tensor("wvt", [D, HPC * HD], DT16, kind="ExternalInput")
    wot = nc.dram_tensor("wot", [D, D], DT16, kind="ExternalInput")
    # plane 0 rows = [A(64); B(64)], plane 1 rows = [C(64); D(64)] so every
    # rotary multiply pairs SBUF operands with equal base partition.
    rq = nc.dram_tensor("rq", [2, 128, BS], DT16, kind="ExternalInput")
    rk = nc.dram_tensor("rk", [2, 128, BS], DT16, kind="ExternalInput")
    out_ext = nc.dram_tensor("out", [SLICE, D], F32, kind="ExternalOutput")

    with tile.TileContext(nc) as tc:
        with tc.tile_pool(name="persist", bufs=1) as pp, \
             tc.tile_pool(name="dramp", bufs=1, space="DRAM") as dramp:
            ident = pp.tile([128, 128], DT16)
            masks.make_identity(nc, ident[:])
            ones_sq = pp.tile([128, 128], DT16)
            nc.gpsimd.memset(ones_sq[:], 1.0)
            eps_t = pp.tile([128, 1], F32)
            nc.gpsimd.memset(eps_t[:], EPS)

            # per-head tensors living through phases 1-2
            qkvp = tc.alloc_tile_pool(name="qkvp", bufs=1)
            q_sb = [qkvp.tile([128, BS], DT16, name=f"q{h}")
                    for h in range(HPC)]
            k_sb = [qkvp.tile([128, BS], DT16, name=f"k{h}")
                    for h in range(HPC)]
            # v in natural layout per 128-row s-tile, ones column at 128
            v_sb = [qkvp.tile([128, BS // 128, HD + 1], BF16, name=f"v{h}")
                    for h in range(HPC)]
            for h in range(HPC):
                nc.gpsimd.memset(v_sb[h][:, :, HD:HD + 1], 1.0)

            # ---------------- Phase 1: QKV + RMS + rotary ----------------
            with tc.tile_pool(name="p1", bufs=1) as p1, \
                 tc.tile_pool(name="p1ps", bufs=1,
                              space=bass.MemorySpace.PSUM) as p1ps:
                wq_s = p1.tile([128, KC, HPC * HD], DT16)
                wk_s = p1.tile([128, KC, HPC * HD], DT16)
                wv_s = p1.tile([128, KC, HPC * HD], DT16)

                def ep_qk(kind, h, ps, rt, s0):
                    dst = (q_sb if kind == "q" else k_sb)[h]
                    sq = p1.tile([128, SCH], DT16, tag="sqv", bufs=3,
                                 name="sq")
                    nc.scalar.square(sq[:], ps[:])
                    ssum = p1ps.tile([128, SCH], F32, tag="ssum", bufs=1,
                                     name="ssum")
                    nc.tensor.matmul(ssum[:], ones_sq[:], sq[:],
                                     start=True, stop=True)
                    # 1/sqrt(var+eps) = exp(-0.5*ln(var+eps)); stays on
                    # ScalarE, avoids the ~2us DVE reciprocal in the chain
                    lnv = p1.tile([128, SCH], F32, tag="sqv", bufs=3,
                                  name="lnv")
                    nc.scalar.activation(
                        lnv[:], ssum[:], mybir.ActivationFunctionType.Ln,
                        bias=eps_t[:], scale=1.0 / HD)
                    rstd = p1.tile([128, SCH], DT16, tag="sqv", bufs=3,
                                   name="rstd")
                    nc.scalar.activation(
                        rstd[:], lnv[:], mybir.ActivationFunctionType.Exp,
                        scale=-0.5)
                    qn = p1.tile([128, SCH], DT16, tag="qn", bufs=2,
                                 name="qn")
                    nc.vector.tensor_mul(qn[:], ps[:], rstd[:])
                    xr, xi = qn[0:64, :], qn[64:128, :]
                    ta = p1.tile([64, SCH], DT16, tag="rot0", bufs=2,
                                 name="ta")
                    tb = p1.tile([64, SCH], DT16, tag="rot1", bufs=2,
                                 name="tb")
                    nc.vector.tensor_mul(ta[:], xr, rt[0:64, 0, :])
                    nc.vector.tensor_mul(tb[:], xi, rt[64:128, 0, :])
                    nc.vector.tensor_sub(dst[0:64, s0:s0 + SCH],
                                         ta[:], tb[:])
                    tc2 = p1.tile([64, SCH], DT16, tag="rot0", bufs=2,
                                  name="tc2")
                    td = p1.tile([64, SCH], DT16, tag="rot1", bufs=2,
                                  name="td")
                    nc.vector.tensor_mul(tc2[:], xr, rt[0:64, 1, :])
                    nc.vector.tensor_mul(td[:], xi, rt[64:128, 1, :])
                    nc.vector.tensor_add(dst[64:128, s0:s0 + SCH],
                                         tc2[:], td[:])

                pend_vtr = []

                def flush_vtr():
                    # v transposes deferred one epilogue further so the PE
                    # never waits on the DVE vt copy
                    while pend_vtr:
                        vt, h, tile0 = pend_vtr.pop(0)
                        for st in range(4):
                            tp = p1ps.tile([128, 128], DT16, tag="vtp",
                                           bufs=2, name="tp")
                            nc.tensor.transpose(
                                tp[:], vt[:, st * 128:(st + 1) * 128],
                                ident[:])
                            nc.vector.tensor_copy(
                                v_sb[h][:, tile0 + st, 0:HD], tp[:])

                def ep_v(h, ps, tile0):
                    vt = p1.tile([128, SCH], DT16, tag="vt", bufs=2,
                                 name="vt")
                    nc.vector.tensor_copy(vt[:], ps[:])
                    pend_vtr.append((vt, h, tile0))

                def p1_epilogue(kind, h, ps, rt, sc):
                    flush_vtr()
                    if kind == "v":
                        ep_v(h, ps, sc * 4)
                    else:
                        ep_qk(kind, h, ps, rt, sc * SCH)

                # 2-deep software pipeline: each output's epilogue is
                # emitted two matmul groups later, giving ScalarE/DVE a
                # full group of slack before the PE consumes their output
                pend = []
                xr_ap = xt.ap().rearrange("(kc p) s -> p kc s", p=128)
                rq_ap = rq.ap().rearrange("f p s -> p f s")
                rk_ap = rk.ap().rearrange("f p s -> p f s")
                for sc in range(NSCH):
                    s0 = sc * SCH
                    # split the 2MB x chunk across both HWDGE queues
                    # (SP + Act) and triple-buffer so the PE never waits
                    xt_t = p1.tile([128, KC, SCH], DT16, tag="xt", bufs=3)
                    nc.sync.dma_start(xt_t[:, 0:8, :],
                                      xr_ap[:, 0:8, s0:s0 + SCH])
                    nc.scalar.dma_start(xt_t[:, 8:16, :],
                                        xr_ap[:, 8:16, s0:s0 + SCH])
                    rq_t = p1.tile([128, 2, SCH], DT16, tag="rq", bufs=2)
                    rk_t = p1.tile([128, 2, SCH], DT16, tag="rk", bufs=2)
                    if sc == 0:
                        # startup: wq right behind chunk 0's x on the two
                        # HWDGE queues (the first matmul groups need only
                        # those); rotary and wk/wv stream in behind,
                        # kc-split across both queues, while the q groups
                        # run (kind-major order below)
                        for wsb_, wsrc in ((wq_s, wqt), (None, None),
                                           (wk_s, wkt), (wv_s, wvt)):
                            if wsb_ is None:
                                nc.sync.dma_start(
                                    rq_t[:], rq_ap[:, :, s0:s0 + SCH])
                                nc.scalar.dma_start(
                                    rk_t[:], rk_ap[:, :, s0:s0 + SCH])
                                continue
                            wr_ = wsrc.ap().rearrange("(kc p) m -> p kc m",
                                                      p=128)
                            nc.sync.dma_start(wsb_[:, 0:8, :],
                                              wr_[:, 0:8, :])
                            nc.scalar.dma_start(wsb_[:, 8:16, :],
                                                wr_[:, 8:16, :])
                    else:
                        nc.sync.dma_start(rq_t[:],
                                          rq_ap[:, :, s0:s0 + SCH])
                        nc.scalar.dma_start(rk_t[:],
                                            rk_ap[:, :, s0:s0 + SCH])

                    if sc == 0 or sc == NSCH - 1:
                        # kind-major: chunk 0 so q groups run while wk/wv
                        # stream in; the last chunk so the final pipeline
                        # flush drains cheap v epilogues, not q/k chains
                        order = [(h, kind) for kind in ("q", "k", "v")
                                 for h in range(HPC)]
                    else:
                        order = [(h, kind) for h in range(HPC)
                                 for kind in ("q", "k", "v")]
                    for h, kind in order:
                        hs = h * HD
                        wsb = {"q": wq_s, "k": wk_s, "v": wv_s}[kind]
                        rt = rq_t if kind == "q" else rk_t
                        ps = p1ps.tile([128, SCH], F32, tag="mm",
                                       bufs=5)
                        for kc in range(KC):
                            nc.tensor.matmul(
                                ps[:], wsb[:, kc, hs:hs + HD],
                                xt_t[:, kc, :],
                                start=(kc == 0), stop=(kc == KC - 1))
                        pend.append((kind, h, ps, rt, sc))
                        if len(pend) > 2:
                            p1_epilogue(*pend.pop(0))
                for item in pend:
                    p1_epilogue(*item)
                flush_vtr()

            # ---------------- Phase 2: attention ----------------
            # 4 passes; pass t computes q-blocks m=2t,2t+1 (512 wide) for
            # every (b,h). Output ownership is stride-4 interleaved so each
            # 512-block contributes one 128-tile to every dest core of its
            # batch; after each pass an AllToAll ships a [2048,256] chunk
            # and the previous pass's output projection runs behind it.
            # last pass ships its two 128-col halves separately so the
            # final AllToAll overlaps the last blocks' compute
            a2a_in_t = [dramp.tile([D, 256], DT16, name=f"a2a_in{t}")
                        for t in range(NT - 1)]
            a2a_out_t = [dramp.tile([D, 256], DT16, name=f"a2a_out{t}")
                         for t in range(NT - 1)]
            a2a_in_h = [dramp.tile([D, 128], DT16, name=f"a2a_inh{u}")
                        for u in range(2)]
            a2a_out_h = [dramp.tile([D, 128], DT16, name=f"a2a_outh{u}")
                         for u in range(2)]
            with tc.tile_pool(name="p2", bufs=1) as p2, \
                 tc.tile_pool(name="p3", bufs=1) as p3, \
                 tc.tile_pool(name="p2ps", bufs=1,
                              space=bass.MemorySpace.PSUM) as p2ps:
                wo_ap = wot.ap().rearrange("(kc p) m -> p kc m", p=128)
                wo_tiles = {}

                def emit_wo_dma(t, n):
                    wt = p3.tile([128, KC, 512], DT16, tag="wo", bufs=3,
                                 name="wo_t")
                    eng = nc.sync if n % 2 == 0 else nc.scalar
                    eng.dma_start(wt[:],
                                  wo_ap[:, :, n * 512:(n + 1) * 512])
                    wo_tiles[(t, n)] = wt

                pend_tr = []

                def flush_tr():
                    # PE part of the previous block's epilogue (transposes
                    # + staging copies), deferred one block so the DVE/Act
                    # normalization chain never stalls the PE
                    while pend_tr:
                        att, att_c, b, h, sub, mpar = pend_tr.pop(0)
                        tp2 = p2ps.tile([128, 128], DT16, tag="aux", bufs=2,
                                        name="tp2")
                        nc.tensor.transpose(tp2[:], att[:], ident[:])
                        nc.vector.tensor_copy(
                            att_c[:, b, h, sub, mpar, :], tp2[:])

                def attention_block512(b, h, m, att_c, mpar):
                    qc = b * S + m * 512
                    # 4 PV accumulators packed 2 per PSUM bank
                    ops = p2ps.tile([128, 4, 256], F32, tag="ops", bufs=1,
                                    name="ops")
                    def emit_pv(pb, kq2):
                        for i in range(2):
                            jt = b * 32 + kq2 * 2 + i
                            for sub in range(4):
                                # start=True clears has_written for the
                                # WHOLE bank, so with 2 accumulators per
                                # bank only the bank-leading sub (0, 2) of
                                # the very first matmul may set it; the
                                # other accumulators' first write lands on
                                # cleared bits and overwrites stale data.
                                nc.tensor.matmul(
                                    ops[:, sub, 0:HD + 1],
                                    pb[:, i, sub * 128:(sub + 1) * 128],
                                    v_sb[h][:, jt, :],
                                    start=(kq2 == 0 and i == 0
                                           and sub % 2 == 0),
                                    stop=(kq2 == 15 and i == 1),
                                    skip_group_check=True)

                    # 2-deep pipeline: PV for pair kq2-2 emitted after
                    # QK/exp of kq2 so ScalarE has slack before PE consumes
                    pending = []
                    for kq2 in range(16):
                        scs = p2ps.tile([128, 2, 512], F32, tag="scs",
                                        bufs=2, name="scs")
                        for i in range(2):
                            kc0 = b * S + (kq2 * 2 + i) * 128
                            nc.tensor.matmul(
                                scs[:, i, :],
                                k_sb[h][:, kc0:kc0 + 128],
                                q_sb[h][:, qc:qc + 512],
                                start=True, stop=True)
                        pb = p2.tile([128, 2, 512], BF16, tag="pb", bufs=4,
                                     name="pb")
                        nc.scalar.activation(
                            pb[:], scs[:],
                            mybir.ActivationFunctionType.Exp, scale=ISQ)
                        pending.append((pb, kq2))
                        if len(pending) > 2:
                            emit_pv(*pending.pop(0))
                        if kq2 == 6:
                            flush_tr()
                    for item in pending:
                        emit_pv(*item)
                    # epilogue: row sums sit at [:, sub, 128]; reciprocal
                    # on DVE straight out of PSUM (ScalarE is the binding
                    # engine in this phase; the deferred transposes absorb
                    # the chain latency)
                    rs = p2.tile([128, 4], F32, tag="rs", bufs=2, name="rs")
                    nc.vector.reciprocal(rs[:], ops[:, :, HD:HD + 1])
                    for sub in range(4):
                        att = p2.tile([128, 128], DT16, tag="att", bufs=8,
                                      name="att")
                        nc.vector.tensor_scalar_mul(
                            att[:], ops[:, sub, 0:HD], rs[:, sub:sub + 1])
                        pend_tr.append((att, att_c, b, h, sub, mpar))

                at_s_tiles = {}

                def load_at_s(t):
                    # prefetch the projection input for pass t as soon as
                    # its AllToAll result can land (emitted early in pass
                    # t+1 so the sync queue isn't clogged by the scatter)
                    at_s = p3.tile([128, KC, 256], DT16, tag="at_s",
                                   bufs=2, name="at_s")
                    nc.sync.dma_start(
                        at_s[:],
                        a2a_out_t[t][:].rearrange(
                            "(kc p) s -> p kc s", p=128))
                    at_s_tiles[t] = (at_s,)

                def load_at_s_half(t, u, eng):
                    ah = p3.tile([128, KC, 128], DT16, tag="at_sh",
                                 bufs=2, name="at_sh")
                    eng.dma_start(
                        ah[:],
                        a2a_out_h[u][:].rearrange(
                            "(kc p) s -> p kc s", p=128))
                    at_s_tiles.setdefault(t, []).append(ah)

                def proj_piece(t, n, st, ats, wt):
                    row0 = t * 256 + st * 128
                    po = p2ps.tile([128, 512], F32, tag="aux",
                                   bufs=2, name="po")
                    for kc in range(KC):
                        if len(ats) == 1:
                            lhs = ats[0][:, kc, st * 128:(st + 1) * 128]
                        else:
                            lhs = ats[st][:, kc, :]
                        nc.tensor.matmul(
                            po[:], lhs, wt[:, kc, :],
                            start=(kc == 0), stop=(kc == KC - 1))
                    ob = p3.tile([128, 512], F32, tag="ob", bufs=2,
                                 name="ob")
                    nc.vector.tensor_copy(ob[:], po[:])
                    nc.sync.dma_start(
                        out_ext.ap()[row0:row0 + 128,
                                     n * 512:(n + 1) * 512], ob[:])

                def proj_run(t, nlist, fire=()):
                    # fire: wo-piece DMAs to emit now; their slot waits
                    # resolve via this proj's own PE progress, so no
                    # engine head-of-line blocks
                    for key in fire:
                        if key[0] < NT and key not in wo_tiles:
                            emit_wo_dma(*key)
                    ats = at_s_tiles[t]
                    for n in nlist:
                        wt = wo_tiles.pop((t, n))
                        for st in range(2):
                            proj_piece(t, n, st, ats, wt)
                    if nlist[-1] == 3:
                        at_s_tiles.pop(t)

                def proj_run_final(t):
                    # interleaved piece order: all but the st=1 tail run
                    # before the last AllToAll lands
                    emit_wo_dma(t, 2)
                    ats = at_s_tiles[t]
                    wts = {n: wo_tiles.pop((t, n)) for n in (0, 1)}
                    order = [(0, 0), (1, 0), (0, 1), (2, 0), (1, 1),
                             (3, 0), (2, 1), (3, 1)]
                    for i, (n, st) in enumerate(order):
                        if i == 2:
                            emit_wo_dma(t, 3)
                        if n not in wts:
                            wts[n] = wo_tiles.pop((t, n))
                        proj_piece(t, n, st, ats, wts[n])
                    at_s_tiles.pop(t)

                def scatter_a2a(t, att_c, mpar=None):
                    # dest core j rows <- its q-tiles, both heads
                    for j in range(NCORES):
                        bb, g = j // 4, j % 4
                        for h in range(HPC):
                            r0 = j * HPC * HD + h * HD
                            if mpar is None:
                                nc.sync.dma_start(
                                    a2a_in_t[t][r0:r0 + HD, :],
                                    att_c[:, bb, h, g, :, :])
                            else:
                                nc.sync.dma_start(
                                    a2a_in_h[mpar][r0:r0 + HD, :],
                                    att_c[:, bb, h, g, mpar, :])
                    if mpar is None:
                        ins, outs = a2a_in_t[t], a2a_out_t[t]
                    else:
                        ins, outs = a2a_in_h[mpar], a2a_out_h[mpar]
                    nc.gpsimd.collective_compute(
                        "AllToAll", mybir.AluOpType.bypass,
                        ins=[ins[:].opt()], outs=[outs[:].opt()],
                        replica_groups=[list(range(NCORES))])

                for t in range(NT):
                    nblk = 0
                    last = t == NT - 1
                    for mpar_o in ((0, 1),) if not last else ((0,), (1,)):
                        att_c = p2.tile([128, B, HPC, 4, 2, 128], DT16,
                                        tag="attc", bufs=2, name="att_c")
                        for b in range(B):
                            for h in range(HPC):
                                for mpar in mpar_o:
                                    attention_block512(b, h, 2 * t + mpar,
                                                       att_c, mpar)
                                    nblk += 1
                                    if t == 0 and nblk == 5:
                                        emit_wo_dma(0, 0)
                                    elif t == 0 and nblk == 7:
                                        emit_wo_dma(0, 1)
                                    elif t > 0 and nblk == 1:
                                        load_at_s(t - 1)
                        # a slice of the previous pass's projection before
                        # the epilogue flush: PE work that covers the last
                        # block's DVE normalization latency
                        if not last:
                            if t > 0:
                                proj_run(t - 1, [0],
                                         fire=((t - 1, 2), (t - 1, 3)))
                            flush_tr()
                            scatter_a2a(t, att_c)
                        elif mpar_o[0] == 0:
                            proj_run(t - 1, [0],
                                     fire=((t - 1, 2), (t - 1, 3)))
                            flush_tr()
                            scatter_a2a(t, att_c, mpar=0)
                            load_at_s_half(t, 0, nc.sync)
                            proj_run(t - 1, [1])
                        else:
                            # second half: its AllToAll is tail-critical,
                            # scatter first, then the rest of proj(t-1)
                            flush_tr()
                            scatter_a2a(t, att_c, mpar=1)
                            proj_run(t - 1, [2, 3],
                                     fire=((t, 0), (t, 1)))
                            load_at_s_half(t, 1, nc.sync)
                    if 0 < t < NT - 1:
                        proj_run(t - 1, [1, 2, 3], fire=((t, 0), (t, 1)))
                proj_run(NT - 1, [0, 1, 2, 3],
                         fire=((NT - 1, 2), (NT - 1, 3)))
            qkvp.release()

    nc.compile()
    return nc


def _prep(inputs):
    x = np.asarray(inputs["x"], np.float32)
    freqs = np.asarray(inputs["freqs_cis"], np.float32)
    wq = np.asarray(inputs["wq"], np.float32)
    wk = np.asarray(inputs["wk"], np.float32)
    wv = np.asarray(inputs["wv"], np.float32)
    wo = np.asarray(inputs["wo"], np.float32)
    nqw = np.asarray(inputs["norm_q_w"], np.float32)
    nkw = np.asarray(inputs["norm_k_w"], np.float32)

    bf = np.float16
    xt = np.ascontiguousarray(x.reshape(BS, D).T).astype(bf)

    # de-interleave rotary pairs within each head's 128 rows
    perm = np.concatenate([np.arange(0, HD, 2), np.arange(1, HD, 2)])
    full_perm = (np.arange(H)[:, None] * HD + perm[None, :]).reshape(-1)
    wq_p = wq[full_perm]
    wk_p = wk[full_perm]

    fr = freqs.reshape(BS, HD)
    cos = np.ascontiguousarray(fr[:, :64].T)   # [64, BS]
    sin = np.ascontiguousarray(fr[:, 64:].T)
    def rot_coefs(w):
        wr = w[0::2][:, None]
        wi = w[1::2][:, None]
        plane0 = np.concatenate([wr * cos, wi * sin], axis=0)   # [128, BS]
        plane1 = np.concatenate([wr * sin, wi * cos], axis=0)
        return np.stack([plane0, plane1]).astype(bf)
    rq = rot_coefs(nqw)
    rk = rot_coefs(nkw)

    wot = np.ascontiguousarray(wo.T).astype(bf)

    in_maps = []
    for c in range(NCORES):
        r0, r1 = c * HPC * HD, (c + 1) * HPC * HD
        in_maps.append({
            "xt": xt,
            "wqt": np.ascontiguousarray(wq_p[r0:r1].T).astype(bf),
            "wkt": np.ascontiguousarray(wk_p[r0:r1].T).astype(bf),
            "wvt": np.ascontiguousarray(wv[r0:r1].T).astype(bf),
            "wot": wot,
            "rq": rq,
            "rk": rk,
        })
    return in_maps


def kernel(**inputs):
    if "nc" not in _CACHE:
        _CACHE["nc"] = _build()
    nc = _CACHE["nc"]
    in_maps = _prep(inputs)
    res = run_bass_kernel_spmd(nc, in_maps, list(range(NCORES)),
                               **_CACHE.get("run_kwargs", {}))
    _CACHE["last_result"] = res
    # core j=b*4+g owns q-tiles {4k+g, k=0..7} of batch b (row block k)
    out = np.empty((B, S, D), np.float32)
    for j in range(NCORES):
        bb, g = j // 4, j % 4
        rj = np.asarray(res.results[j]["out"]).reshape(8, 128, D)
        for k in range(8):
            t0 = (4 * k + g) * 128
            out[bb, t0:t0 + 128, :] = rj[k]
    return out
